# revision 1
# baseline (speedup 1.0000x reference)
"""MoE (top-k of 8 experts) Trainium2 kernel.

Strategy (expert parallelism per the sharding hint, with balanced
cross-core expert splitting):
  - Host computes the (tiny) gating: logits = x @ Wg, softmax, top-k,
    renormalized combine weights (decisions in float64; verified stable:
    min rank2/rank3 prob gap ~1e-5 >> fp32 noise for these inputs).
  - Host packs the (expert, token) work into 16 uniform slots: each of
    the 8 cores runs two "mega-tiles" of sizes (SA, SB) (1152 + 1024
    for the reference loads).  Each slot holds tokens of exactly ONE
    expert, and that expert's weights are shipped as per-mega inputs,
    so an overloaded expert spills onto another core's slot.  This cuts
    per-core capacity to ~(total_assignments/8) rounded up (2176 vs a
    naive per-expert capacity of 2304).
  - Core kernel (per mega): y = gelu_tanh(X @ W1 + b1) @ W2 * w[:,None]
    with float16 matmuls (inputs rounded to fp16; PE multiplies at
    >=fp16 precision and accumulates fp32; measured end-to-end rel err
    4.2e-4).  fp16 runs at the full 1 row/cycle PE rate and keeps the
    LDWEIGHTS stream fully hidden (FWL), unlike fp32/fp32r.
  - Host scatter-adds the (at most top_k) expert contributions per
    token, plus the combine-weighted b2 term, into the [B, S, D] output.

Device kernel layout (per core, per mega):
  xT [D, C] tokens transposed (d on partitions), loaded per t-slice.
  GEMM1: HactT[h, t] = W1_blk.T @ xT (PSUM-accumulate over d), ACT
         applies gelu_apprx_tanh(z + b1) PSUM->SBUF (fp16 out).
  GEMM2: Y[t, d] = HactT_blk.T @ W2_blk (PSUM-accumulate over the
         1024-row weight block, DVE-add into per-ts SBUF accumulators
         across the 4 weight blocks).
  Y is scaled per-token (tensor_scalar per-partition scalar) and stored
  per-ts so the tail pipelines.  Weights stream from HBM in 1024-row
  blocks (W1 double buffered), once per mega.

Measured on HW: ~505 us/NEFF; tensor-engine matmul stream has zero idle
gaps and runs at ~98% of the 1 row/cycle @2.4GHz floor for this shape.
"""

import os
import numpy as np

D = 1024
H = 4096
E = 8
N_CORES = 8
HBLK = 1024          # h rows per streamed weight block
HB = H // HBLK       # 4 blocks
KD = D // 128        # 8 k-tiles for GEMM1
KHB = HBLK // 128    # 8 k-tiles per block for GEMM2


def _slice_period(n):
    # fp16 matmul issue period (measured): N/2.4GHz + ~3ns dispatch,
    # with a ~100ns floor where the FWL LDWEIGHTS (~97ns) stops being
    # hidden by the moving-operand stream.
    return max(n / 2.4 + 3.0, 100.0)


def _best_slices(mega):
    """DP: split mega into moving-dim slices (multiples of 64, <=512)
    minimizing the summed matmul issue period."""
    best = {0: (0.0, ())}
    for m in range(64, mega + 64, 64):
        cands = []
        for s in range(64, min(512, m) + 64, 64):
            if m - s in best:
                c, parts = best[m - s]
                cands.append((c + _slice_period(s), parts + (s,)))
        if cands:
            best[m] = min(cands)
    assert mega in best, f"no slice decomposition for {mega}"
    _, parts = best[mega]
    out = []
    off = 0
    for s in parts:
        out.append((off, s))
        off += s
    return out


_KERNEL_CACHE = {}
LAST_EXEC_NS = None


def _build_kernel(megas):
    """megas: tuple of mega-tile sizes (each a multiple of 128)."""
    import concourse.bacc as bacc
    import concourse.mybir as mybir
    import concourse.tile as tile

    f32 = mybir.dt.float32
    f16 = mybir.dt.float16
    GELU = mybir.ActivationFunctionType.Gelu_apprx_tanh

    C = sum(megas)
    nc = bacc.Bacc("TRN2", target_bir_lowering=False, debug=False,
                   num_devices=N_CORES)

    # host-swizzled layouts matching the SBUF tile layouts, so each DMA
    # is 128 rows of long contiguous runs (fast descriptor issue):
    #   xT[p, (c kk-interleaved per t-slice)], w1[p, hb, kk, hw],
    #   w2[p, hb, kh, d]
    xT = nc.dram_tensor("xT", [128, C * KD], f16, kind="ExternalInput").ap()
    wts = []
    for mi in range(len(megas)):
        wts.append((
            nc.dram_tensor(f"w1{mi}", [128, HB, KD, HBLK], f16,
                           kind="ExternalInput").ap(),
            nc.dram_tensor(f"w2{mi}", [128, HB, KHB, D], f16,
                           kind="ExternalInput").ap(),
            # pre-transposed on host: [128, H/128], col j = b1[j*128 + p]
            nc.dram_tensor(f"b1{mi}", [128, H // 128], f32,
                           kind="ExternalInput").ap(),
        ))
    # pre-transposed on host: [128, C/128]
    wt = nc.dram_tensor("wt", [128, C // 128], f32,
                        kind="ExternalInput").ap()
    # mega0's entire W1 block 0 pre-staged as two contiguous fast-issue
    # chunks: the first real matmuls wait only on the 512KB "a" chunk
    # (h-tiles 0-1), whose ~8us of matmuls cover the 1.5MB "b" chunk
    # (h-tiles 2-7); block0 then covers all later weight streaming.
    w1h0a = nc.dram_tensor("w1h0a", [128, KD * 256], f16,
                           kind="ExternalInput").ap()
    w1h0b = nc.dram_tensor("w1h0b", [128, KD * 768], f16,
                           kind="ExternalInput").ap()
    y = nc.dram_tensor("y", [C, D], f32, kind="ExternalOutput").ap()

    with tile.TileContext(nc) as tc:
        with (
            tc.tile_pool(name="meta", bufs=1) as pmeta,
            tc.tile_pool(name="xg", bufs=3) as pxg,
            tc.tile_pool(name="yacc", bufs=10) as pyacc,
            tc.tile_pool(name="w1p", bufs=2) as pw1,
            tc.tile_pool(name="w2p", bufs=1) as pw2,
            tc.tile_pool(name="hact", bufs=1) as phact,
            tc.tile_pool(name="ps1", bufs=4, space="PSUM") as pps1,
            tc.tile_pool(name="ps2", bufs=4, space="PSUM") as pps2,
        ):
            y_r = y.rearrange("(t p) d -> p t d", p=128)
            wtt = None

            # PE warmup on zeros during the DMA head: holds the HAM
            # clock-gate at 2.4GHz before the first real matmul.
            # memset on DVE so the gpsimd queue stays free for the
            # critical x-slice DMAs; 8 matmuls end just as the first
            # real operands land (~9.5us).
            warm = pmeta.tile([128, 512], f16, name="warm")
            nc.vector.memset(warm[:], 0.0)
            for wi in range(8):
                pw = pps1.tile([128, 512], f32, tag="ps1",
                               name=f"warm_ps_{wi}")
                nc.tensor.matmul(pw[:], warm[:, :128], warm[:],
                                 start=True, stop=True)

            w1h0a_t = pmeta.tile([128, KD, 256], f16, name="w1h0a_t")
            nc.sync.dma_start(
                w1h0a_t[:], w1h0a.rearrange("p (kk h) -> p kk h", kk=KD))
            w1h0b_t = pmeta.tile([128, KD, 768], f16, name="w1h0b_t")
            nc.sync.dma_start(
                w1h0b_t[:], w1h0b.rearrange("p (kk h) -> p kk h", kk=KD))

            off = 0
            for mi, mega in enumerate(megas):
                w1d, w2d, b1d = wts[mi]
                ts_count = mega // 128
                ts0 = off // 128
                sl = _best_slices(mega)

                # per-slice token loads (pipelines the kernel head;
                # gpsimd queue so they don't serialize behind weights)
                xgs = []
                for (soff, slen) in sl:
                    xg = pxg.tile([128, KD, slen], f16, tag="xgs",
                                  name=f"xg_{mi}_{soff}")
                    base = (off + soff) * KD
                    nc.gpsimd.dma_start(
                        xg[:],
                        xT[:, base:base + slen * KD]
                        .rearrange("p (kk c) -> p kk c", kk=KD))
                    xgs.append(xg)

                b1t = pmeta.tile([128, H // 128], f32, tag=f"b1_{mi}")
                nc.sync.dma_start(b1t[:], b1d[:])
                if wtt is None:
                    wtt = pmeta.tile([128, C // 128], f32, name="wtt")
                    nc.sync.dma_start(wtt[:], wt[:])

                yas = [pyacc.tile([128, D], f32, tag="ya",
                                  name=f"ya_{mi}_{ts}")
                       for ts in range(ts_count)]

                for hb in range(HB):
                    if mi == 0 and hb == 0:
                        w1t = None   # served from w1h0a_t / w1h0b_t
                    else:
                        w1t = pw1.tile([128, KD, HBLK], f16, tag="w1t")
                        nc.sync.dma_start(w1t[:], w1d[:, hb, :, :])
                    ht = phact.tile([128, KHB, mega], f16, tag="ht")

                    # GEMM1 + gelu. For the very first block, loop
                    # hs-outer so the pre-staged w1h0 (h-tiles 0-1)
                    # covers ~8us of matmuls while the full W1 block
                    # DMA completes; elsewhere slice-outer pipelines
                    # the x loads.
                    first_blk = mi == 0 and hb == 0
                    if first_blk:
                        # data-arrival order: (s0,s1) x h-tiles 0-1 need
                        # only the small w1h0a chunk + first two x
                        # slices; h-tiles 2-7 wait on w1h0b; the last x
                        # slice (s2) arrives last.
                        ns = len(sl)
                        order = ([(si, hs) for si in range(min(2, ns))
                                  for hs in range(2)]
                                 + [(si, hs) for si in range(min(2, ns))
                                    for hs in range(2, KHB)]
                                 + [(si, hs) for si in range(2, ns)
                                    for hs in range(KHB)])
                    else:
                        order = [(si, hs) for si in range(len(sl))
                                 for hs in range(KHB)]
                    for si, hs in order:
                        soff, slen = sl[si]
                        if True:
                            ps = pps1.tile([128, 512], f32, tag="ps1")
                            for k in range(KD):
                                if first_blk:
                                    w1s = (
                                        w1h0a_t[:, k, hs * 128:(hs + 1) * 128]
                                        if hs < 2 else
                                        w1h0b_t[:, k,
                                                (hs - 2) * 128:(hs - 1) * 128])
                                else:
                                    w1s = w1t[:, k, hs * 128:(hs + 1) * 128]
                                nc.tensor.matmul(
                                    ps[:, :slen],
                                    w1s,
                                    xgs[si][:, k, :],
                                    start=(k == 0), stop=(k == KD - 1),
                                )
                            nc.scalar.activation(
                                ht[:, hs, soff:soff + slen], ps[:, :slen],
                                GELU,
                                bias=b1t[:, hb * KHB + hs:hb * KHB + hs + 1],
                            )

                    # W2 block load deferred past GEMM1 in program order
                    w2t = pw2.tile([128, KHB, D], f16, tag="w2t")
                    nc.sync.dma_start(w2t[:], w2d[:, hb, :, :])

                    # GEMM2 partial: Y[t, d] += Hact_blk.T @ W2_blk
                    for ts in range(ts_count):
                        for dh in range(2):
                            ps2 = pps2.tile([128, 512], f32, tag="ps2")
                            for k in range(KHB):
                                nc.tensor.matmul(
                                    ps2[:],
                                    ht[:, k, ts * 128:(ts + 1) * 128],
                                    w2t[:, k, dh * 512:(dh + 1) * 512],
                                    start=(k == 0), stop=(k == KHB - 1),
                                )
                            dst = yas[ts][:, dh * 512:(dh + 1) * 512]
                            if hb == 0:
                                nc.vector.tensor_copy(dst, ps2[:])
                            else:
                                nc.vector.tensor_add(dst, dst, ps2[:])
                        if hb == HB - 1:
                            # scale + store as soon as a ts finishes
                            nc.vector.tensor_scalar_mul(
                                yas[ts][:], yas[ts][:],
                                wtt[:, ts0 + ts:ts0 + ts + 1])
                            nc.gpsimd.dma_start(
                                y_r[:, ts0 + ts, :], yas[ts][:])

                off += mega

    nc.compile()
    return nc


def _get_kernel(megas):
    megas = tuple(megas)
    if megas not in _KERNEL_CACHE:
        _KERNEL_CACHE[megas] = _build_kernel(megas)
    return _KERNEL_CACHE[megas]


def _route(xt, Wg, top_k):
    logits = xt.astype(np.float64) @ Wg.astype(np.float64)
    m = logits.max(axis=-1, keepdims=True)
    p = np.exp(logits - m)
    p /= p.sum(axis=-1, keepdims=True)
    order = np.argsort(-p, axis=-1, kind="stable")
    idx = order[:, :top_k]
    vals = np.take_along_axis(p, idx, axis=-1)
    w = vals / vals.sum(axis=-1, keepdims=True)
    return idx, w


def _pack(loads):
    """Pick uniform per-core mega sizes (SA, SB) and assign each expert
    exactly two slots (possibly on different cores): sorted by load
    desc, the k largest experts get two A slots, the middle get (A, B),
    the k smallest get two B slots.  Returns (SA, SB, assign) where
    assign is [(expert, [("A"|"B", core), ...]), ...]."""
    order = np.argsort(-loads, kind="stable")
    ls = loads[order]
    best = None
    for Ctot in range(2048, 4096 + 1, 128):
        for SA in range((Ctot + 255) // 256 * 128, Ctot - 511, 128):
            SB = Ctot - SA
            if SB < 512 or SB > SA:
                continue
            for k in range(0, 5):
                nmid = E - 2 * k
                if nmid < 0:
                    continue
                ok = (all(ls[i] <= 2 * SA for i in range(k))
                      and all(ls[i] <= SA + SB for i in range(k, k + nmid))
                      and all(ls[i] <= 2 * SB for i in range(k + nmid, E)))
                if ok:
                    best = (SA, SB, k)
                    break
            if best:
                break
        if best:
            break
    assert best is not None, f"no packing for loads {loads}"
    SA, SB, k = best
    # slot assignment
    slotsA = list(range(E))        # one A slot per core
    slotsB = list(range(E))        # one B slot per core
    assign = []                    # (expert, [slots...]) slot=(core, which)
    ai = bi = 0
    for i in range(E):
        e = order[i]
        if i < k:
            s = [("A", slotsA[ai]), ("A", slotsA[ai + 1])]
            ai += 2
        elif i < k + (E - 2 * k):
            s = [("A", slotsA[ai]), ("B", slotsB[bi])]
            ai += 1
            bi += 1
        else:
            s = [("B", slotsB[bi]), ("B", slotsB[bi + 1])]
            bi += 2
        assign.append((e, s))
    return SA, SB, assign


def kernel(x, Wg, W1, b1, W2, b2, top_k):
    import concourse.bass_utils as bass_utils

    top_k = int(top_k)
    B, S, d = x.shape
    T = B * S
    xt = np.ascontiguousarray(np.asarray(x, dtype=np.float32).reshape(T, d))
    Wg = np.asarray(Wg, dtype=np.float32)
    W1 = np.asarray(W1, dtype=np.float32)
    b1 = np.asarray(b1, dtype=np.float32)
    W2 = np.asarray(W2, dtype=np.float32)
    b2 = np.asarray(b2, dtype=np.float32)

    idx, w = _route(xt, Wg, top_k)
    # swizzle weights to the device DMA layouts (see _build_kernel)
    W1h = np.ascontiguousarray(
        W1.astype(np.float16)
        .reshape(E, KD, 128, HB, HBLK).transpose(0, 2, 3, 1, 4))
    W2h = np.ascontiguousarray(
        W2.astype(np.float16)
        .reshape(E, HB, KHB, 128, D).transpose(0, 3, 1, 2, 4))
    b1h = np.ascontiguousarray(
        b1.reshape(E, H // 128, 128).transpose(0, 2, 1))

    toks = []
    wts_host = []
    for e in range(E):
        hit = idx == e
        sel = np.nonzero(hit.any(axis=1))[0]
        pos = np.argmax(hit[sel], axis=1)
        we = np.take_along_axis(w[sel], pos[:, None], axis=1)[:, 0]
        toks.append(sel)
        wts_host.append(we.astype(np.float32))
    loads = np.array([len(t) for t in toks])

    SA, SB, assign = _pack(loads)
    megas = (SA, SB)
    C = SA + SB
    nc = _get_kernel(megas)

    # build per-core inputs; slot bookkeeping for the scatter phase
    xTe = [np.zeros((128, KD, C), dtype=np.float16) for _ in range(N_CORES)]
    wte = [np.zeros((C,), dtype=np.float32) for _ in range(N_CORES)]
    wmaps = [{} for _ in range(N_CORES)]
    scatter = []   # (core, mega_off, n, token_indices)
    for e, slots in assign:
        pos = 0
        for which, core in slots:
            cap = SA if which == "A" else SB
            moff = 0 if which == "A" else SA
            n = min(cap, len(toks[e]) - pos)
            if n > 0:
                tk = toks[e][pos:pos + n]
                xTe[core][:, :, moff:moff + n] = (
                    xt[tk].astype(np.float16)
                    .reshape(n, KD, 128).transpose(2, 1, 0))
                wte[core][moff:moff + n] = wts_host[e][pos:pos + n]
                scatter.append((core, moff, n, tk))
                pos += n
            mi = 0 if which == "A" else 1
            wmaps[core][f"w1{mi}"] = W1h[e]
            wmaps[core][f"w2{mi}"] = W2h[e]
            wmaps[core][f"b1{mi}"] = b1h[e]
            if mi == 0:
                wmaps[core]["w1h0a"] = np.ascontiguousarray(
                    W1h[e][:, 0, :, :256]).reshape(128, -1)
                wmaps[core]["w1h0b"] = np.ascontiguousarray(
                    W1h[e][:, 0, :, 256:]).reshape(128, -1)
        assert pos == len(toks[e]), f"expert {e} tokens not fully placed"

    # flatten x into the per-slice kk-interleaved DMA layout
    slice_spans = []
    off0 = 0
    for mega in megas:
        for (soff, slen) in _best_slices(mega):
            slice_spans.append((off0 + soff, slen))
        off0 += mega
    in_maps = []
    for c in range(N_CORES):
        xdev = np.empty((128, C * KD), dtype=np.float16)
        for (a, slen) in slice_spans:
            xdev[:, a * KD:(a + slen) * KD] = (
                xTe[c][:, :, a:a + slen].reshape(128, -1))
        m = {"xT": xdev,
             "wt": np.ascontiguousarray(wte[c].reshape(C // 128, 128).T)}
        # default weights for any unused slot (keep NEFF inputs bound)
        for mi in range(2):
            if f"w1{mi}" not in wmaps[c]:
                wmaps[c][f"w1{mi}"] = W1h[0]
                wmaps[c][f"w2{mi}"] = W2h[0]
                wmaps[c][f"b1{mi}"] = b1h[0]
                if mi == 0:
                    wmaps[c]["w1h0a"] = np.ascontiguousarray(
                        W1h[0][:, 0, :, :256]).reshape(128, -1)
                    wmaps[c]["w1h0b"] = np.ascontiguousarray(
                        W1h[0][:, 0, :, 256:]).reshape(128, -1)
        m.update(wmaps[c])
        in_maps.append(m)

    trace = os.environ.get("MOE_TRACE", "") not in ("", "0")
    run_kwargs = {}
    if trace:
        _install_ntff_hook()
        run_kwargs = dict(
            trace=True,
            trace_cores=[int(c) for c in
                         os.environ.get("MOE_TRACE_CORES", "0").split(",")],
            tmpdir=os.environ.get("MOE_TRACE_DIR") or None,
        )
    res = bass_utils.run_bass_kernel_spmd(
        nc, in_maps, core_ids=list(range(N_CORES)), **run_kwargs)
    if trace:
        global LAST_EXEC_NS
        LAST_EXEC_NS = res.exec_time_ns
        print(f"MOE exec_time_ns: {res.exec_time_ns}")
        if res.instructions_and_trace:
            print(f"MOE trace: {res.instructions_and_trace[1]}")

    out = np.zeros((T, D), dtype=np.float32)
    for core, moff, n, tk in scatter:
        out[tk] += res.results[core]["y"][moff:moff + n]
    combine = np.zeros((T, E), dtype=np.float32)
    np.put_along_axis(combine, idx, w.astype(np.float32), axis=1)
    out += combine @ b2

    return out.reshape(B, S, d).astype(np.float32)


def _install_ntff_hook():
    import sys, types
    if "antenv.axon_hooks" in sys.modules:
        return
    mod = types.ModuleType("antenv.axon_hooks")
    store = {"h": None}
    mod.set_axon_ntff_profile_hook = lambda h: store.__setitem__("h", h)
    mod.get_axon_ntff_profile_hook = lambda: store["h"]
    import antenv
    sys.modules["antenv.axon_hooks"] = mod
    antenv.axon_hooks = mod
    try:
        from trn_agent_boot.trn_boot import _ntff_profile_via_ctypes
        mod.set_axon_ntff_profile_hook(
            _ntff_profile_via_ctypes("/opt/axon/libaxon_pjrt.so"))
    except Exception as exc:
        print(f"ntff hook install failed: {exc}")



# revision 3
# speedup vs baseline: 1.0259x; 1.0259x over previous
"""MoE (top-k of 8 experts) Trainium2 kernel — H-sharded expert parallelism.

Strategy (v2): instead of sharding tokens across cores (which strands
capacity on load imbalance), shard each expert's FFN hidden dimension:
core c owns H-columns [512c, 512c+512) of W1 and H-rows of W2 for ALL
8 experts.  Every core processes every (token, expert) assignment for
its H-slice:

  - per-core work = sum_e L_e = top_k * T tokens exactly (16384) --
    PERFECT balance for any routing, zero padding: the PE stream is
    the top-2 flop floor, 64 cycles/token/core = 1.049M cyc = 437us.
  - weights: each core holds 1/8th of every expert = 16.8 MB/core,
    exactly one copy of all weights cluster-wide (half of v1).
  - x arena: tokens packed per-expert (dup for top-2) -- IDENTICAL
    for all cores (only weights differ per core).
  - y: cores produce partial sums over their H-slice; host adds the 8
    partials (fp16 partials, fp32 accumulate), scales by the top-k
    combine weights, scatter-adds, and adds the combine@b2 term.

Device kernel (per core), per expert-slot of L tokens, slices <=512:
  GEMM1: ps[h128, t] += W1slc[d128, h128].T @ xT[d128, t] over KD=8
         k-tiles; ACT applies gelu_tanh(ps + b1) -> ht fp16.
  GEMM2 (flipped, moving=tokens so ragged L costs exactly L cycles):
         ps[d128, t] += W2slc[h128, d128].T @ ht[h128, t] over KH=4
         k-tiles; DVE copies psum -> fp16 y tile; store [dh, t] runs.
  Units are software-pipelined: G1(i) ... G2(i-1) so the ACT latency
  hides under the next unit's GEMM1.
"""

import os
import numpy as np

D = 1024
H = 4096
E = 8
N_CORES = 8
HSL = H // N_CORES       # per-core H slice (512)
KD = D // 128            # 8 k-tiles for GEMM1
KH = HSL // 128          # 4 k-tiles for GEMM2
DH = D // 128            # 8 output d-tiles for GEMM2

_KERNEL_CACHE = {}
LAST_EXEC_NS = None


def _unit_list(slot_sizes):
    """Token-slice decomposition: per slot, ~equal slices <=512; the
    very first units form a ramp of small slices: the kernel head is
    HBM-bound (all 8 cores fetch their first x slices at once), so
    small first units let the PE start at ~3us and stay fed while the
    prefetch pipeline fills.  The last slot ends with a small slice so
    the PE tail is short.
    Returns list of (mi, soff, slen, xcol) plus arena cols."""
    units = []
    xcol = 0
    ramp = [128, 128, 128, 192, 256, 320, 448]
    for mi, L in enumerate(slot_sizes):
        def balanced(n_tokens):
            if n_tokens == 0:
                return []
            n = (n_tokens + 511) // 512
            base, rem = divmod(n_tokens, n)
            return [base + 1] * rem + [base] * (n - rem)
        if mi == 0 and L > sum(ramp) + 256:
            sizes = list(ramp) + balanced(L - sum(ramp))
        else:
            sizes = balanced(L)
        if mi == len(slot_sizes) - 1 and sizes[-1] > 256:
            sizes = sizes[:-1] + [sizes[-1] - 128, 128]
        soff = 0
        for s in sizes:
            units.append((mi, soff, s, xcol))
            soff += s
            xcol += KD * s
        assert soff == L
    return units, xcol


def _build_kernel(slot_sizes):
    """slot_sizes: tuple of per-expert token counts (same across cores;
    cores differ only in which H-slice of the weights they receive)."""
    import concourse.bacc as bacc
    import concourse.mybir as mybir
    import concourse.tile as tile

    f32 = mybir.dt.float32
    f16 = mybir.dt.float16
    GELU = mybir.ActivationFunctionType.Gelu_apprx_tanh

    units, XCOL = _unit_list(slot_sizes)
    NS = len(slot_sizes)
    nu = len(units)

    nc = bacc.Bacc("TRN2", target_bir_lowering=False, debug=False,
                   num_devices=N_CORES)

    xT = nc.dram_tensor("xT", [128, XCOL], f16, kind="ExternalInput").ap()
    w1d, w2d, b1d = [], [], []
    for m in range(NS):
        if m == 0:
            # slot 0 W1 hs-major / W2 dh-major so the head streams in
            # small contiguous chunks the first (ramp) units can chase
            w1d.append(nc.dram_tensor("w10", [128, KH, KD, 128], f16,
                                      kind="ExternalInput").ap())
            w2d.append(nc.dram_tensor("w20", [128, DH, KH, 128], f16,
                                      kind="ExternalInput").ap())
        else:
            w1d.append(nc.dram_tensor(f"w1{m}", [128, KD, HSL], f16,
                                      kind="ExternalInput").ap())
            w2d.append(nc.dram_tensor(f"w2{m}", [128, KH, D], f16,
                                      kind="ExternalInput").ap())
        b1d.append(nc.dram_tensor(f"b1{m}", [128, KH], f32,
                                  kind="ExternalInput").ap())
    y = nc.dram_tensor("y", [128, XCOL], f16, kind="ExternalOutput").ap()

    with tile.TileContext(nc) as tc:
        with (
            tc.tile_pool(name="meta", bufs=1) as pmeta,
            # bufs=3 doubles as the x-prefetch throttle: slice i+2's DMA
            # can only issue once slice i's tile is released, so head
            # DMAs can't pile up and saturate HBM.
            tc.tile_pool(name="xg", bufs=3) as pxg,
            tc.tile_pool(name="xk", bufs=2 * KD) as pxk,
            tc.tile_pool(name="w1p", bufs=2) as pw1,
            tc.tile_pool(name="w2p", bufs=3) as pw2,
            tc.tile_pool(name="b1p", bufs=3) as pb1,
            tc.tile_pool(name="htp", bufs=4) as pht,
            tc.tile_pool(name="ytp", bufs=6) as pyt,
            tc.tile_pool(name="ps1", bufs=3, space="PSUM") as pps1,
            tc.tile_pool(name="ps2", bufs=5, space="PSUM") as pps2,
        ):
            # PE warmup on zeros: hold the HAM clock gate at 2.4GHz
            # through the head DMAs.
            warm = pmeta.tile([128, 512], f16, name="warm")
            nc.vector.memset(warm[:], 0.0)
            for wi in range(12):
                pw = pps1.tile([128, 512], f32, tag="ps1",
                               name=f"warm_ps_{wi}")
                nc.tensor.matmul(pw[:], warm[:, :128], warm[:],
                                 start=True, stop=True)

            w1t = {}   # slot -> sbuf tile (chunk list for slot 0)
            w2t = {}
            b1t = {}

            def load_head_weights():
                # critical chain on the sync HWDGE queue (its own FIFO):
                # b1, then slot-0 W1 as 8 k-chunks (0.125MB each) that
                # gemm1(0)'s k-outer loop consumes as they land.
                bt = pb1.tile([128, KH], f32, tag="b1t")
                nc.sync.dma_start(bt[:], b1d[0][:])
                b1t[0] = bt
                chunks = []
                for hs in range(KH):
                    c = pmeta.tile([128, KD, 128], f16, name=f"w1c0_{hs}")
                    nc.sync.dma_start(c[:], w1d[0][:, hs])
                    chunks.append(c)
                w1t[0] = chunks
                # slot-0 W2 as 8 dh-major chunks on scalar (behind x0):
                # gemm2(0) consumes them in order as they land
                c2 = []
                for dh in range(DH):
                    c = pmeta.tile([128, KH, 128], f16, name=f"w2c0_{dh}")
                    nc.scalar.dma_start(c[:], w2d[0][:, dh])
                    c2.append(c)
                w2t[0] = c2

            def load_w1(m):
                if m >= NS:
                    return
                t = pw1.tile([128, KD, HSL], f16, tag="w1t")
                nc.gpsimd.dma_start(t[:], w1d[m][:])
                w1t[m] = t
                bt = pb1.tile([128, KH], f32, tag="b1t")
                nc.gpsimd.dma_start(bt[:], b1d[m][:])
                b1t[m] = bt

            def load_w2(m):
                if m >= NS:
                    return
                t2 = pw2.tile([128, KH, D], f16, tag="w2t")
                nc.gpsimd.dma_start(t2[:], w2d[m][:])
                w2t[m] = t2

            def w1slice(m, k, hs):
                if m == 0:
                    return w1t[0][hs][:, k, :]
                return w1t[m][:, k, hs * 128:(hs + 1) * 128]

            def w2slice(m, k, dh):
                if m == 0:
                    return w2t[0][dh][:, k, :]
                return w2t[m][:, k, dh * 128:(dh + 1) * 128]

            xgs = {}
            KSPLIT = 0   # per-k x streaming for head units (disabled:
                         # the small-unit ramp won on measurement)

            def load_x(i):
                if i >= nu:
                    return
                mi, soff, slen, xc = units[i]
                if i < KSPLIT:
                    # head units: 8 matched k-chunks on a fast HWDGE
                    # ring; gemm1's k-outer loop chases the arrivals
                    eng = nc.scalar if i == 0 else nc.sync
                    chunks = []
                    for k in range(KD):
                        t = pxk.tile([128, slen], f16, tag="xk",
                                     name=f"xk_{i}_{k}")
                        eng.dma_start(
                            t[:], xT[:, xc + k * slen:xc + (k + 1) * slen])
                        chunks.append(t)
                    xgs[i] = chunks
                    return
                xg = pxg.tile([128, KD, slen], f16, tag="xg",
                              name=f"xg_{i}")
                # unit 2 rides the scalar HWDGE queue (behind w2c0); the
                # rest go on gpsimd where the xg pool bufs throttle
                # prefetch so head DMAs don't saturate HBM.
                eng = nc.scalar if i < 3 else nc.gpsimd
                eng.dma_start(
                    xg[:],
                    xT[:, xc:xc + KD * slen]
                    .rearrange("p (kk c) -> p kk c", kk=KD))
                xgs[i] = xg

            hts = {}

            def gemm1(i):
                mi, soff, slen, xc = units[i]
                ht = pht.tile([128, KH, slen], f16, tag="ht",
                              name=f"ht_{i}")
                if i < KSPLIT:
                    # k-outer: consume (x, W1) k-chunk pairs as they
                    # arrive; 4 psum tiles accumulate across the k loop
                    pss = [pps1.tile([128, 512], f32, tag="ps1",
                                     name=f"ps1k_{i}_{hs}")
                           for hs in range(KH)]
                    for k in range(KD):
                        for hs in range(KH):
                            nc.tensor.matmul(
                                pss[hs][:, :slen], w1slice(mi, k, hs),
                                xgs[i][k][:, :slen],
                                start=(k == 0), stop=(k == KD - 1))
                    for hs in range(KH):
                        nc.scalar.activation(
                            ht[:, hs, :slen], pss[hs][:, :slen], GELU,
                            bias=b1t[mi][:, hs:hs + 1])
                else:
                    for hs in range(KH):
                        ps = pps1.tile([128, 512], f32, tag="ps1")
                        for k in range(KD):
                            nc.tensor.matmul(
                                ps[:, :slen], w1slice(mi, k, hs),
                                xgs[i][:, k, :slen],
                                start=(k == 0), stop=(k == KD - 1))
                        nc.scalar.activation(
                            ht[:, hs, :slen], ps[:, :slen], GELU,
                            bias=b1t[mi][:, hs:hs + 1])
                hts[i] = ht

            def gemm2(i):
                mi, soff, slen, xc = units[i]
                yt = pyt.tile([128, DH, slen], f16, tag="yt",
                              name=f"yt_{i}")
                for dh in range(DH):
                    ps = pps2.tile([128, 512], f32, tag="ps2")
                    for k in range(KH):
                        nc.tensor.matmul(
                            ps[:, :slen],
                            w2slice(mi, k, dh),
                            hts[i][:, k, :slen],
                            start=(k == 0), stop=(k == KH - 1))
                    nc.vector.tensor_copy(yt[:, dh, :slen], ps[:, :slen])
                    if i >= nu - 2 and dh in (3, 7):
                        # final unit: store in two halves so the last
                        # DMA (and its completion receipt) is tiny
                        eng = nc.scalar if dh == 3 else nc.sync
                        h0 = 0 if dh == 3 else 1
                        eng.dma_start(
                            y[:, xc + h0 * 4 * slen:xc + (h0 + 1) * 4 * slen]
                            .rearrange("p (dh c) -> p dh c", dh=4),
                            yt[:, h0 * 4:(h0 + 1) * 4, :])
                if i < nu - 2:
                    # alternate the two HWDGE queues so store backlog on
                    # one can't stall the pipeline
                    eng = nc.scalar if (i % 2 == 0) else nc.sync
                    eng.dma_start(
                        y[:, xc:xc + DH * slen]
                        .rearrange("p (dh c) -> p dh c", dh=DH),
                        yt[:])
                del hts[i]
                del xgs[i]

            # Gate tiles: DMAs are spread across 8 independent HW lanes,
            # so program order cannot delay slot>=1 weight prefetch (3MB)
            # out of the HBM-bound head -- only data dependencies can.
            # Gates occupy BOTH w1/w2 pool slots (same tag) and are
            # released by a tiny DVE read of an early unit's gelu
            # output, so the weight DMAs can only issue after the head
            # congestion has cleared.
            def emit_gate(pool, shape, tag, src_tile):
                g = pool.tile(shape, f16, tag=tag)
                nc.vector.tensor_copy(g[:1, 0, :1], src_tile[:1, 0, :1])

            slot1_start = nu
            for i, u in enumerate(units):
                if u[0] == 1:
                    slot1_start = i
                    break
            # gates release after these units' gelu; slot-1 weights load
            # one unit before slot 1 begins
            gate_at = (max(1, slot1_start - 3), max(2, slot1_start - 2))
            w1_1_at = max(3, slot1_start - 1)
            use_gates = NS > 1 and w1_1_at > gate_at[1]
            g2_next = 0
            for i, u in enumerate(units):
                if i == 0:
                    # x first on the scalar HWDGE ring (its FIFO), then
                    # the slot-0 weight chunks
                    load_x(0)
                    load_x(1)
                    load_x(2)
                    load_head_weights()
                elif u[0] != units[i - 1][0]:
                    load_w1(u[0] + 1)
                    load_w2(u[0] + 1)
                if i >= 1:
                    load_x(i + 2)
                gemm1(i)
                if use_gates and i in gate_at:
                    emit_gate(pw1, [128, KD, HSL], "w1t", hts[i])
                    emit_gate(pw2, [128, KH, D], "w2t", hts[i])
                if NS > 1 and i == min(w1_1_at, slot1_start - 1, nu - 1):
                    load_w1(1)
                    load_w2(1)
                # gemm2 lags gemm1 by 2 units at the head (buys the
                # chunked w2c0 time to land), by 1 in steady state (keeps
                # the kernel tail short)
                lag = 2 if i < 9 else 1
                while g2_next <= i - lag:
                    gemm2(g2_next)
                    g2_next += 1
            while g2_next < nu:
                gemm2(g2_next)
                g2_next += 1

    nc.compile()
    return nc


def _get_kernel(slot_sizes):
    slot_sizes = tuple(slot_sizes)
    if slot_sizes not in _KERNEL_CACHE:
        _KERNEL_CACHE[slot_sizes] = _build_kernel(slot_sizes)
    return _KERNEL_CACHE[slot_sizes]


def _route(xt, Wg, top_k):
    logits = xt.astype(np.float64) @ Wg.astype(np.float64)
    m = logits.max(axis=-1, keepdims=True)
    p = np.exp(logits - m)
    p /= p.sum(axis=-1, keepdims=True)
    order = np.argsort(-p, axis=-1, kind="stable")
    idx = order[:, :top_k]
    vals = np.take_along_axis(p, idx, axis=-1)
    w = vals / vals.sum(axis=-1, keepdims=True)
    return idx, w


def kernel(x, Wg, W1, b1, W2, b2, top_k):
    import concourse.bass_utils as bass_utils

    top_k = int(top_k)
    B, S, d = x.shape
    T = B * S
    xt = np.ascontiguousarray(np.asarray(x, dtype=np.float32).reshape(T, d))
    Wg = np.asarray(Wg, dtype=np.float32)
    W1 = np.asarray(W1, dtype=np.float32)
    b1 = np.asarray(b1, dtype=np.float32)
    W2 = np.asarray(W2, dtype=np.float32)
    b2 = np.asarray(b2, dtype=np.float32)

    idx, w = _route(xt, Wg, top_k)
    toks = []
    wts_host = []
    for e in range(E):
        hit = idx == e
        sel = np.nonzero(hit.any(axis=1))[0]
        pos = np.argmax(hit[sel], axis=1)
        we = np.take_along_axis(w[sel], pos[:, None], axis=1)[:, 0]
        toks.append(sel)
        wts_host.append(we.astype(np.float32))
    loads = np.array([len(t) for t in toks])

    # slots: experts by descending load (first slot largest, last smallest)
    slot_order = [int(e) for e in np.argsort(-loads, kind="stable")
                  if loads[e] > 0]
    slot_sizes = tuple(int(loads[e]) for e in slot_order)
    nc = _get_kernel(slot_sizes)
    units, XCOL = _unit_list(slot_sizes)

    # x arena (identical across cores): per slot, tokens ascending;
    # per unit, [128, KD, slen] flattened.
    xarena = np.empty((128, XCOL), dtype=np.float16)
    xslots = {}
    for m, e in enumerate(slot_order):
        xe = xt[toks[e]].astype(np.float16)          # [L, 1024]
        xslots[m] = np.ascontiguousarray(
            xe.reshape(-1, KD, 128).transpose(2, 1, 0))  # [128, KD, L]
    for (mi, soff, slen, xc) in units:
        xarena[:, xc:xc + KD * slen] = (
            xslots[mi][:, :, soff:soff + slen].reshape(128, -1))

    # per-core weight slices: core c owns H-cols [512c, 512c+512)
    W1h = np.ascontiguousarray(
        W1.astype(np.float16)
        .reshape(E, KD, 128, N_CORES, HSL).transpose(3, 0, 2, 1, 4))
    # -> [c, e, 128, KD, HSL]
    W2h = np.ascontiguousarray(
        W2.astype(np.float16)
        .reshape(E, N_CORES, KH, 128, D).transpose(1, 0, 3, 2, 4))
    # -> [c, e, 128, KH, D]
    b1h = np.ascontiguousarray(
        b1.reshape(E, N_CORES, KH, 128).transpose(1, 0, 3, 2))
    # -> [c, e, 128, KH]

    in_maps = []
    for c in range(N_CORES):
        mmap = {"xT": xarena}
        for m, e in enumerate(slot_order):
            if m == 0:
                # slot 0 is hs-major W1 / dh-major W2 (see _build_kernel)
                mmap["w10"] = np.ascontiguousarray(
                    W1h[c, e].reshape(128, KD, KH, 128)
                    .transpose(0, 2, 1, 3))
                mmap["w20"] = np.ascontiguousarray(
                    W2h[c, e].reshape(128, KH, DH, 128)
                    .transpose(0, 2, 1, 3))
            else:
                mmap[f"w1{m}"] = W1h[c, e]
                mmap[f"w2{m}"] = W2h[c, e]
            mmap[f"b1{m}"] = b1h[c, e]
        in_maps.append(mmap)

    trace = os.environ.get("MOE_TRACE", "") not in ("", "0")
    run_kwargs = {}
    if trace:
        _install_ntff_hook()
        run_kwargs = dict(
            trace=True,
            trace_cores=[int(c) for c in
                         os.environ.get("MOE_TRACE_CORES", "0").split(",")],
            tmpdir=os.environ.get("MOE_TRACE_DIR") or None,
        )
    res = bass_utils.run_bass_kernel_spmd(
        nc, in_maps, core_ids=list(range(N_CORES)), **run_kwargs)
    if trace:
        global LAST_EXEC_NS
        LAST_EXEC_NS = res.exec_time_ns
        print(f"MOE exec_time_ns: {res.exec_time_ns}")
        if res.instructions_and_trace:
            print(f"MOE trace: {res.instructions_and_trace[1]}")

    # host reduction: sum per-core H-slice partials (fp32), then scale
    # by combine weights and scatter-add; add the combine @ b2 term.
    ysum = res.results[0]["y"].astype(np.float32)
    for c in range(1, N_CORES):
        ysum += res.results[c]["y"]

    out = np.zeros((T, D), dtype=np.float32)
    for (mi, soff, slen, xc) in units:
        e = slot_order[mi]
        blk = (ysum[:, xc:xc + DH * slen].reshape(128, DH, slen)
               .transpose(2, 1, 0).reshape(slen, D))   # [t, d]
        tk = toks[e][soff:soff + slen]
        out[tk] += wts_host[e][soff:soff + slen, None] * blk
    combine = np.zeros((T, E), dtype=np.float32)
    np.put_along_axis(combine, idx, w.astype(np.float32), axis=1)
    out += combine @ b2

    return out.reshape(B, S, d).astype(np.float32)


def _install_ntff_hook():
    import sys, types
    if "antenv.axon_hooks" in sys.modules:
        return
    mod = types.ModuleType("antenv.axon_hooks")
    store = {"h": None}
    mod.set_axon_ntff_profile_hook = lambda h: store.__setitem__("h", h)
    mod.get_axon_ntff_profile_hook = lambda: store["h"]
    import antenv
    sys.modules["antenv.axon_hooks"] = mod
    antenv.axon_hooks = mod
    try:
        from trn_agent_boot.trn_boot import _ntff_profile_via_ctypes
        mod.set_axon_ntff_profile_hook(
            _ntff_profile_via_ctypes("/opt/axon/libaxon_pjrt.so"))
    except Exception as exc:
        print(f"ntff hook install failed: {exc}")
